# revision 9
# baseline (speedup 1.0000x reference)
"""nn_Encoder_48095043780825: 2-hop weighted-mean SAGEConv GNN encoder
on 8 Trainium2 NeuronCores (Bass/Tile), self-contained.

kernel(**inputs) -> np.ndarray [200000, 128] float32

Distribution (per-input JIT-specialized program; integer structure baked,
float math on device):
 - Host relabels the 50k nodes, balanced by in-degree, into 8 cores x 49
   blocks of 128 slots (dst-sharding).  Edges are partitioned by destination
   block and packed into 128-edge chunks; edge weights are pre-normalized on
   host (w / max(segsum(w,dst),eps)) so no on-device denominator pass.
 - Gathers are BATCHED via dma_gather (InstDMAGatherAnt, mlp GPSIMD ucode
   library): ONE instruction fetches up to 64 chunks x 128 rows, amortizing
   the ~1us SWDGE per-instruction overhead ~64x vs per-chunk indirect DMAs.
   dma_gather indices are int16 (<32768), so chunks are packed
   half-homogeneously: srcs < 32768 gather from table[:32768] (stream A),
   the rest from table[32768:] (stream B, base-offset AP).
 - Datapath is bf16 (tables, messages, masks, weights); PSUM accumulation
   fp32.  Per chunk, DVE builds mask[e,dst]=wn_e*(iota==dst_e) (bf16) and the
   PE accumulates agg_fm += msg.T @ mask in PSUM -- feature-major agg, no
   transpose needed before the W matmuls.
 - Per block: hx = Ws.T@xT_blk + Wn.T@aggT (PE, bf16), bias+ReLU on ACT into
   the feature-major master xT; PE-transpose + ACT copy to node-major; DMA to
   the local shard of the next hop's gather table.
 - Single AllGather (x1 only): the final phase is sharded by node-OWNER core,
   so x2 gathers read the core's local 6272-row shard - no second AllGather
   and single-stream int16 indices.
 - Final phase: batched dma_gathers of x2[node] and effect_emb[effect] rows
   (bf16), one DVE add per group, partition-major fp32 output tiles DMA'd
   contiguously; host unsorts rows.
"""
import sys
sys.path.insert(0, "/opt/trn_rl_repo")
import heapq
import numpy as np

import jax
from jax.sharding import Mesh, PartitionSpec
from jax.experimental.shard_map import shard_map

from concourse import bass, mybir, library_config
from concourse.tile import TileContext
from concourse.bass2jax import (
    _bass_exec_p,
    install_neuronx_cc_hook,
    partition_id_tensor,
)

import ml_dtypes

P = 128
F32 = mybir.dt.float32
BF16 = mybir.dt.bfloat16
I32 = mybir.dt.int32
I16 = mybir.dt.int16
BF = ml_dtypes.bfloat16
HALF = 32768

CFG = dict(N=50000, E=600000, D=128, NEFF=1000, Q=200000, C=8, B=49, HOPS=2)

KG = 64   # chunks per hop gather group (KG*128 rows per dma_gather)
KF = 32   # chunks per final-phase gather group


# ------------------------------------------------------------------ wait split

def _split_wide_waits(nc, max_waits=1):
    """This walrus build rejects instructions with more than one sync-wait
    command; move excess waits onto preceding NoOps on the same engine."""
    for f in nc.m.functions:
        for bb in f.blocks:
            new_instrs = []
            for ins in bb.instructions:
                si = ins.sync_info
                if si is not None and si.on_wait and len(si.on_wait) > max_waits:
                    waits = list(si.on_wait)
                    head, tail = waits[:-max_waits], waits[-max_waits:]
                    for i in range(0, len(head), max_waits):
                        nop = mybir.InstNoOp(
                            name=nc.get_next_instruction_name(),
                            engine=ins.engine,
                            ins=[], outs=[],
                            sync_info=mybir.SyncInfo(
                                on_wait=head[i:i + max_waits], on_update=[]),
                            text_hint="waitsplit",
                            bass_nofuse=True,
                        )
                        new_instrs.append(nop)
                    si.on_wait = tail
                new_instrs.append(ins)
            bb.instructions = new_instrs


# ------------------------------------------------------------------ host prep

def _balance_nodes(dst, N, n_bins):
    npad = n_bins * P
    deg = np.bincount(dst, minlength=N).astype(np.int64)
    deg_pad = np.zeros(npad, np.int64)
    deg_pad[:N] = deg
    order = np.argsort(-deg_pad, kind="stable")
    heap = [(0, b) for b in range(n_bins)]
    heapq.heapify(heap)
    counts = np.zeros(n_bins, np.int32)
    newid = np.empty(npad, np.int64)
    for n in order:
        while True:
            load, b = heapq.heappop(heap)
            if counts[b] < P:
                break
        newid[n] = b * P + counts[b]
        counts[b] += 1
        if counts[b] < P:
            heapq.heappush(heap, (load + deg_pad[n], b))
    assert counts.min() == counts.max() == P
    return newid


def _wrap_idx16(idx, nch):
    """idx[i] -> [16g + i%16, i//16] for g in 0..7 (replica per Q7 core).
    Returns [P, nch*8] int16."""
    ni = nch * P
    full = np.zeros(ni, np.int16)
    full[:len(idx)] = idx
    arr = np.zeros((P, ni // 16), np.int16)
    blk = full.reshape(ni // 16, 16).T
    for g in range(8):
        arr[16 * g:16 * (g + 1)] = blk
    return arr


def _prep(inputs, cfg):
    N, E, D, NEFF, Q = cfg["N"], cfg["E"], cfg["D"], cfg["NEFF"], cfg["Q"]
    C, B = cfg["C"], cfg["B"]
    NPC = B * P
    NPAD = C * NPC
    assert NPAD >= N and D == 128

    graph_x = np.asarray(inputs["graph_x"], np.float32)
    edge_index = np.asarray(inputs["edge_index"])
    src = edge_index[0].astype(np.int64)
    dst = edge_index[1].astype(np.int64)
    w = np.asarray(inputs["chemical_similarity"], np.float32)
    x_nodes = np.asarray(inputs["x_nodes"]).astype(np.int64)
    effect_ids = np.asarray(inputs["effect_ids"]).astype(np.int64)
    W_self = np.asarray(inputs["W_self"], np.float32)
    W_neigh = np.asarray(inputs["W_neigh"], np.float32)
    bias = np.asarray(inputs["bias"], np.float32)
    effect_emb = np.asarray(inputs["effect_emb"], np.float32)

    newid = _balance_nodes(dst, N, C * B)

    x_full = np.zeros((NPAD, D), np.float32)
    x_full[newid[:N]] = graph_x

    # pre-normalized edge weights (reference: agg/max(segsum(w,dst),1e-12))
    denom = np.zeros(N, np.float64)
    np.add.at(denom, dst, w.astype(np.float64))
    wn = (w.astype(np.float64)
          / np.maximum(denom[dst], 1e-12)).astype(np.float32)

    nsrc = newid[src]
    ndst = newid[dst]
    ecore = ndst // NPC
    eblk = (ndst % NPC) // P
    eslot = ndst % P
    ehalf = (nsrc >= HALF).astype(np.int64)

    loads = np.zeros((C, B, 2), np.int64)
    np.add.at(loads, (ecore, eblk, ehalf), 1)
    # uniform chunk counts per (block, stream) across cores
    CS_b = -(-loads.max(axis=0) // P)          # [B, 2]
    zero = CS_b.sum(axis=1) == 0
    CS_b[zero, 0] = 1
    C_b = CS_b.sum(axis=1)                      # chunks per block
    NCH = int(C_b.sum())
    cob = np.concatenate([[0], np.cumsum(C_b)])
    # stream-position bases per block
    sposA0 = np.concatenate([[0], np.cumsum(CS_b[:, 0])])
    sposB0 = np.concatenate([[0], np.cumsum(CS_b[:, 1])])
    NCHA, NCHB = int(sposA0[-1]), int(sposB0[-1])

    # per-chunk metadata (same for all cores): stream + stream position
    chunk_stream = np.zeros(NCH, np.int64)
    chunk_spos = np.zeros(NCH, np.int64)
    for b in range(B):
        ca, cbn = int(CS_b[b, 0]), int(CS_b[b, 1])
        j0 = int(cob[b])
        chunk_stream[j0:j0 + ca] = 0
        chunk_spos[j0:j0 + ca] = sposA0[b] + np.arange(ca)
        chunk_stream[j0 + ca:j0 + ca + cbn] = 1
        chunk_spos[j0 + ca:j0 + ca + cbn] = sposB0[b] + np.arange(cbn)

    edstf = np.zeros((C, P, NCH), np.float32)
    ewf = np.zeros((C, P, NCH), np.float32)
    idxA = np.zeros((C, NCHA * P), np.int16)
    idxB = np.zeros((C, NCHB * P), np.int16)

    # vectorized packing: edges sorted by (core, block, half), rank in group
    eorder = np.lexsort((ehalf, eblk, ecore))
    ec, eb, es = ecore[eorder], eblk[eorder], ehalf[eorder]
    gstart = np.concatenate(
        [[0], np.cumsum(loads.ravel())[:-1]]).reshape(C, B, 2)
    rank = np.arange(E, dtype=np.int64) - gstart[ec, eb, es]
    cig = rank // P                             # chunk index within group
    slot = rank % P
    jglob = cob[eb] + np.where(es == 0, cig, CS_b[eb, 0] + cig)
    spos = np.where(es == 0, sposA0[eb] + cig, sposB0[eb] + cig)
    edstf[ec, slot, jglob] = eslot[eorder].astype(np.float32)
    ewf[ec, slot, jglob] = wn[eorder]
    sidx = spos * P + slot
    vals = (nsrc[eorder] - es * HALF).astype(np.int16)
    mA = es == 0
    idxA[ec[mA], sidx[mA]] = vals[mA]
    idxB[ec[~mA], sidx[~mA]] = vals[~mA]

    # queries sharded by node-owner core
    qnew = newid[x_nodes]
    qc = (qnew // NPC).astype(np.int64)
    qrow = (qnew % NPC).astype(np.int64)
    nq = np.bincount(qc, minlength=C)
    QCH = int(-(-nq.max() // P))
    qnode = np.zeros((C, QCH * P), np.int16)
    qeff = np.zeros((C, QCH * P), np.int16)
    qpos = []
    qorder = np.lexsort((np.arange(Q), qc))
    qstart = np.concatenate([[0], np.cumsum(nq)])
    for c in range(C):
        sel = qorder[qstart[c]:qstart[c + 1]]
        qpos.append(sel)
        qnode[c, :len(sel)] = qrow[sel].astype(np.int16)
        qeff[c, :len(sel)] = effect_ids[sel].astype(np.int16)

    iF = np.tile(np.arange(P, dtype=BF)[None, :], (P, 1))
    ident = np.eye(P, dtype=BF)

    meta = dict(cfg, NPC=NPC, NPAD=NPAD, NCH=NCH, NCHA=NCHA, NCHB=NCHB,
                QCH=QCH, C_b=C_b, CS_b=CS_b, chunk_of_block=cob,
                chunk_stream=chunk_stream, chunk_spos=chunk_spos,
                qpos=qpos, nq=nq)

    x0b = x_full.astype(BF)
    in_maps = []
    for c in range(C):
        in_maps.append({
            "x0b": x0b,
            "xT0": x_full[c * NPC:(c + 1) * NPC].T.astype(BF).copy(),
            "effb": effect_emb.astype(BF),
            "idxA": _wrap_idx16(idxA[c], NCHA),
            "idxB": _wrap_idx16(idxB[c], max(1, NCHB)),
            "edst": edstf[c], "ewn": ewf[c],
            "qnode16": _wrap_idx16(qnode[c], QCH),
            "qeff16": _wrap_idx16(qeff[c], QCH),
            "iF": iF, "ident": ident,
            "Wsb": W_self.astype(BF), "Wnb": W_neigh.astype(BF),
            "biasc": bias.T.astype(np.float32).copy(),
        })
    return meta, in_maps


# --------------------------------------------------------------- device build

def _build_nc(meta, krep=1, no_collective=False):
    C, B, D, NEFF = meta["C"], meta["B"], meta["D"], meta["NEFF"]
    NPC, NPAD, NCH = meta["NPC"], meta["NPAD"], meta["NCH"]
    NCHA, NCHB, QCH = meta["NCHA"], meta["NCHB"], meta["QCH"]
    C_b, CS_b, cob = meta["C_b"], meta["CS_b"], meta["chunk_of_block"]
    chunk_stream, chunk_spos = meta["chunk_stream"], meta["chunk_spos"]
    HOPS = meta["HOPS"]
    NCHBp = max(1, NCHB)

    nc = bass.Bass(trn_type="TRN2", num_devices=C, num_swdge_queues=2)

    x0b = nc.dram_tensor("x0b", [NPAD, D], BF16, kind="ExternalInput")
    xT0 = nc.dram_tensor("xT0", [D, NPC], BF16, kind="ExternalInput")
    effb = nc.dram_tensor("effb", [NEFF, D], BF16, kind="ExternalInput")
    idxA = nc.dram_tensor("idxA", [P, NCHA * 8], I16, kind="ExternalInput")
    idxB = nc.dram_tensor("idxB", [P, NCHBp * 8], I16, kind="ExternalInput")
    edst = nc.dram_tensor("edst", [P, NCH], F32, kind="ExternalInput")
    ewn = nc.dram_tensor("ewn", [P, NCH], F32, kind="ExternalInput")
    qnode16 = nc.dram_tensor("qnode16", [P, QCH * 8], I16,
                             kind="ExternalInput")
    qeff16 = nc.dram_tensor("qeff16", [P, QCH * 8], I16,
                            kind="ExternalInput")
    iFd = nc.dram_tensor("iF", [P, P], BF16, kind="ExternalInput")
    identd = nc.dram_tensor("ident", [P, P], BF16, kind="ExternalInput")
    Wsb = nc.dram_tensor("Wsb", [HOPS, D, D], BF16, kind="ExternalInput")
    Wnb = nc.dram_tensor("Wnb", [HOPS, D, D], BF16, kind="ExternalInput")
    biasc = nc.dram_tensor("biasc", [D, HOPS], F32, kind="ExternalInput")

    newx = [nc.dram_tensor(f"newx{h}", [NPC, D], BF16) for h in range(HOPS)]
    x1_full = nc.dram_tensor("x1_full", [NPAD, D], BF16, addr_space="Shared")
    out_pm = nc.dram_tensor("out_pm", [P, QCH, D], F32, kind="ExternalOutput")

    rg = [list(range(C))]
    gq = [0]

    def gath(pool, table_ap, idx_sb, spos0, nco, kmax, tag):
        """One dma_gather of nco chunks (nco*128 rows) starting at stream
        position spos0; idx slice cols [spos0*8, (spos0+nco)*8)."""
        t = pool.tile([P, kmax, D], BF16, tag=tag)
        ni = nco * P
        nc.gpsimd.dma_gather(
            t[:, :nco, :], table_ap,
            idx_sb[:, spos0 * 8:(spos0 + nco) * 8],
            ni, ni, D, elem_step=D,
            queue_num=gq[0] % 2, single_packet=False)
        gq[0] += 1
        return t

    with TileContext(nc) as tc:
        nc.gpsimd.load_library(library_config.mlp)
        with tc.tile_pool(name="const", bufs=1) as cp:
            iF = cp.tile([P, P], BF16)
            nc.sync.dma_start(out=iF[:], in_=iFd[:, :])
            idn = cp.tile([P, P], BF16)
            nc.sync.dma_start(out=idn[:], in_=identd[:, :])
            Ws, Wn = [], []
            for h in range(HOPS):
                t = cp.tile([P, D], BF16, tag=f"ws{h}")
                nc.sync.dma_start(out=t[:], in_=Wsb[h, :, :])
                Ws.append(t)
                t = cp.tile([P, D], BF16, tag=f"wn{h}")
                nc.sync.dma_start(out=t[:], in_=Wnb[h, :, :])
                Wn.append(t)
            bc = cp.tile([P, HOPS], F32)
            nc.sync.dma_start(out=bc[:], in_=biasc[:, :])
            edst_sb = cp.tile([P, NCH], F32)
            nc.sync.dma_start(out=edst_sb[:], in_=edst[:, :])
            ewn_sb = cp.tile([P, NCH], F32)
            nc.sync.dma_start(out=ewn_sb[:], in_=ewn[:, :])
            idxA_sb = cp.tile([P, NCHA * 8], I16, tag="idxA")
            nc.sync.dma_start(out=idxA_sb[:], in_=idxA[:, :])
            idxB_sb = cp.tile([P, NCHBp * 8], I16, tag="idxB")
            nc.sync.dma_start(out=idxB_sb[:], in_=idxB[:, :])
            qnode_sb = cp.tile([P, QCH * 8], I16, tag="qn")
            nc.sync.dma_start(out=qnode_sb[:], in_=qnode16[:, :])
            qeff_sb = cp.tile([P, QCH * 8], I16, tag="qe")
            nc.sync.dma_start(out=qeff_sb[:], in_=qeff16[:, :])
            xT0_sb = cp.tile([P, NPC], BF16, tag="xT0")
            nc.sync.dma_start(out=xT0_sb[:], in_=xT0[:, :])
            xT1_sb = cp.tile([P, NPC], BF16, tag="xT1")

            for rep in range(krep):
                for h in range(HOPS):
                    table = x0b if h == 0 else x1_full
                    tabA = table[:HALF, :]
                    tabB = table[HALF:, :]
                    xT_cur = xT0_sb if h == 0 else xT1_sb
                    with tc.tile_pool(name=f"ga{h}_{rep}", bufs=2) as gpa, \
                         tc.tile_pool(name=f"gb{h}_{rep}", bufs=2) as gpb, \
                         tc.tile_pool(name=f"m{h}_{rep}", bufs=8) as mp, \
                         tc.tile_pool(name=f"s{h}_{rep}", bufs=4) as sp, \
                         tc.tile_pool(name=f"pa{h}_{rep}", bufs=2,
                                      space="PSUM") as pa, \
                         tc.tile_pool(name=f"ph{h}_{rep}", bufs=2,
                                      space="PSUM") as ph:
                        gtile = [None, None]
                        gcur = [-1, -1]
                        for b in range(B):
                            nchunks = int(C_b[b])
                            agg_ps = pa.tile([P, P], F32, tag="agg",
                                             space="PSUM")
                            for k in range(nchunks):
                                j = int(cob[b]) + k
                                s = int(chunk_stream[j])
                                spos = int(chunk_spos[j])
                                g = spos // KG
                                if g != gcur[s]:
                                    gcur[s] = g
                                    if s == 0:
                                        nco = min(KG, NCHA - g * KG)
                                        gtile[0] = gath(
                                            gpa, tabA, idxA_sb, g * KG,
                                            nco, KG, "msgA")
                                    else:
                                        nco = min(KG, NCHB - g * KG)
                                        gtile[1] = gath(
                                            gpb, tabB, idxB_sb, g * KG,
                                            nco, KG, "msgB")
                                msg = gtile[s][:, spos - g * KG, :]
                                mask = mp.tile([P, P], BF16, tag="mask")
                                nc.vector.tensor_scalar(
                                    out=mask[:], in0=iF[:],
                                    scalar1=edst_sb[:, j:j + 1],
                                    scalar2=ewn_sb[:, j:j + 1],
                                    op0=mybir.AluOpType.is_equal,
                                    op1=mybir.AluOpType.mult)
                                nc.tensor.matmul(
                                    agg_ps[:], lhsT=msg, rhs=mask[:],
                                    start=(k == 0), stop=(k == nchunks - 1))
                            aggT = sp.tile([P, P], BF16, tag="aggT")
                            nc.scalar.activation(
                                aggT[:], agg_ps[:],
                                mybir.ActivationFunctionType.Copy)
                            hx_ps = ph.tile([P, P], F32, tag="hx",
                                            space="PSUM")
                            nc.tensor.matmul(
                                hx_ps[:], lhsT=Ws[h][:],
                                rhs=xT_cur[:, b * P:(b + 1) * P],
                                start=True, stop=False)
                            nc.tensor.matmul(hx_ps[:], lhsT=Wn[h][:],
                                             rhs=aggT[:],
                                             start=False, stop=True)
                            if h == 0:
                                fm_dst = xT1_sb[:, b * P:(b + 1) * P]
                            else:
                                fm_t = sp.tile([P, P], BF16, tag="fm")
                                fm_dst = fm_t[:]
                            nc.scalar.activation(
                                fm_dst, hx_ps[:],
                                mybir.ActivationFunctionType.Relu,
                                bias=bc[:, h:h + 1])
                            nx_ps = ph.tile([P, P], BF16, tag="nxp",
                                            space="PSUM")
                            nc.tensor.transpose(out=nx_ps[:], in_=fm_dst,
                                                identity=idn[:])
                            nx = sp.tile([P, P], BF16, tag="nx")
                            nc.scalar.activation(
                                nx[:], nx_ps[:],
                                mybir.ActivationFunctionType.Copy)
                            nc.sync.dma_start(
                                out=newx[h][b * P:(b + 1) * P, :], in_=nx[:])
                    if h == 0:
                        if no_collective:
                            nc.sync.dma_start(
                                out=x1_full[0:NPC, :], in_=newx[0][:, :])
                        else:
                            nc.gpsimd.collective_compute(
                                "AllGather", mybir.AluOpType.bypass,
                                replica_groups=rg,
                                ins=[newx[0][:]],
                                outs=[x1_full[:]])

                with tc.tile_pool(name=f"fx_{rep}", bufs=2) as fx, \
                     tc.tile_pool(name=f"fe_{rep}", bufs=2) as fe, \
                     tc.tile_pool(name=f"fo_{rep}", bufs=2) as fo:
                    for g0 in range(0, QCH, KF):
                        nco = min(KF, QCH - g0)
                        xt = gath(fx, newx[HOPS - 1][:, :], qnode_sb, g0,
                                  nco, KF, "x2g")
                        et = gath(fe, effb[:, :], qeff_sb, g0,
                                  nco, KF, "efg")
                        ot = fo.tile([P, KF, D], F32, tag="ot")
                        nc.vector.tensor_tensor(
                            out=ot[:, :nco, :], in0=xt[:, :nco, :],
                            in1=et[:, :nco, :],
                            op=mybir.AluOpType.add)
                        nc.sync.dma_start(
                            out=out_pm[:, g0:g0 + nco, :],
                            in_=ot[:, :nco, :])
    return nc


# ------------------------------------------------------------------- runner

def _build_runner(nc, n_cores):
    install_neuronx_cc_hook()
    partition_name = nc.partition_id_tensor.name if nc.partition_id_tensor else None

    in_names, out_names, out_avals = [], [], []
    for alloc in nc.m.functions[0].allocations:
        if not isinstance(alloc, mybir.MemoryLocationSet):
            continue
        name = alloc.memorylocations[0].name
        if alloc.kind == "ExternalInput":
            if name != partition_name:
                in_names.append(name)
        elif alloc.kind == "ExternalOutput":
            out_names.append(name)
            out_avals.append(jax.core.ShapedArray(
                tuple(alloc.tensor_shape), mybir.dt.np(alloc.dtype)))

    n_params = len(in_names)
    n_outs = len(out_avals)
    all_in_names = list(in_names) + list(out_names)
    if partition_name is not None:
        all_in_names.append(partition_name)

    def _body(*args):
        operands = list(args)
        if partition_name is not None:
            operands.append(partition_id_tensor())
        outs = _bass_exec_p.bind(
            *operands,
            out_avals=tuple(out_avals),
            in_names=tuple(all_in_names),
            out_names=tuple(out_names),
            lowering_input_output_aliases=(),
            sim_require_finite=True,
            sim_require_nnan=True,
            nc=nc,
        )
        return tuple(outs)

    devices = jax.devices()[:n_cores]
    mesh = Mesh(np.asarray(devices), ("core",))
    in_specs = (PartitionSpec("core"),) * (n_params + n_outs)
    out_specs = (PartitionSpec("core"),) * n_outs
    sharded = jax.jit(
        shard_map(_body, mesh=mesh, in_specs=in_specs, out_specs=out_specs,
                  check_rep=False),
        keep_unused=True,
    )

    def make_args(in_maps):
        per_core = [[np.asarray(m[name]) for name in in_names] for m in in_maps]
        concat_in = [
            np.concatenate([per_core[c][i] for c in range(n_cores)], axis=0)
            for i in range(n_params)
        ]
        concat_zeros = [
            np.zeros((n_cores * av.shape[0], *av.shape[1:]), av.dtype)
            for av in out_avals
        ]
        from jax.sharding import NamedSharding
        sh = NamedSharding(mesh, PartitionSpec("core"))
        return [jax.device_put(a, sh) for a in concat_in + concat_zeros]

    def run(in_maps):
        args = make_args(in_maps)
        out_arrs = sharded(*args)
        jax.block_until_ready(out_arrs)
        return [
            {name: np.asarray(out_arrs[i]).reshape(
                n_cores, *out_avals[i].shape)[c]
             for i, name in enumerate(out_names)}
            for c in range(n_cores)
        ]

    def timeit(in_maps, reps=1):
        import time
        args = make_args(in_maps)
        out = sharded(*args)
        jax.block_until_ready(out)
        ts = []
        for _ in range(reps):
            t0 = time.perf_counter()
            out = sharded(*args)
            jax.block_until_ready(out)
            ts.append(time.perf_counter() - t0)
        return out, ts

    return run, timeit


# ------------------------------------------------------------------- kernel

def kernel(**inputs):
    gx = np.asarray(inputs["graph_x"])
    cfg = dict(
        N=gx.shape[0],
        E=np.asarray(inputs["edge_index"]).shape[1],
        D=gx.shape[1],
        NEFF=np.asarray(inputs["effect_emb"]).shape[0],
        Q=np.asarray(inputs["x_nodes"]).shape[0],
        C=8,
        B=-(-gx.shape[0] // (8 * P)),
        HOPS=np.asarray(inputs["W_self"]).shape[0],
    )
    meta, in_maps = _prep(inputs, cfg)
    nc = _build_nc(meta)
    _split_wide_waits(nc, 1)
    mybir.codegen_inst_isa_subclasses(nc)
    run, _ = _build_runner(nc, cfg["C"])
    results = run(in_maps)

    C, D, Q, QCH = cfg["C"], cfg["D"], cfg["Q"], meta["QCH"]
    out = np.empty((Q, D), np.float32)
    for c in range(C):
        pm = results[c]["out_pm"]                       # [P, QCH, P]
        vals = pm.transpose(1, 0, 2).reshape(QCH * P, D)
        sel = meta["qpos"][c]
        out[sel] = vals[:len(sel)]
    return out


# revision 10
# speedup vs baseline: 1.3635x; 1.3635x over previous
"""nn_Encoder_48095043780825: 2-hop weighted-mean SAGEConv GNN encoder
on 8 Trainium2 NeuronCores (Bass/Tile), self-contained.

kernel(**inputs) -> np.ndarray [200000, 128] float32

Distribution (per-input JIT-specialized program; integer structure baked,
float math on device):
 - Host relabels the 50k nodes, balanced by in-degree, into 8 cores x 49
   blocks of 128 slots (dst-sharding).  Edges are partitioned by destination
   block and packed into 128-edge chunks; edge weights are pre-normalized on
   host (w / max(segsum(w,dst),eps)) so no on-device denominator pass.
 - Gathers are BATCHED via dma_gather (InstDMAGatherAnt, mlp GPSIMD ucode
   library): ONE instruction fetches up to 64 chunks x 128 rows, amortizing
   the ~1us SWDGE per-instruction overhead ~64x vs per-chunk indirect DMAs.
   dma_gather indices are int16 (<32768), so chunks are packed
   half-homogeneously: srcs < 32768 gather from table[:32768] (stream A),
   the rest from table[32768:] (stream B, base-offset AP).
 - Datapath is bf16 (tables, messages, masks, weights); PSUM accumulation
   fp32.  Per chunk, DVE builds mask[e,dst]=wn_e*(iota==dst_e) (bf16) and the
   PE accumulates agg_fm += msg.T @ mask in PSUM -- feature-major agg, no
   transpose needed before the W matmuls.
 - Per block: hx = Ws.T@xT_blk + Wn.T@aggT (PE, bf16), bias+ReLU on ACT into
   the feature-major master xT; PE-transpose + ACT copy to node-major; DMA to
   the local shard of the next hop's gather table.
 - Single AllGather (x1 only): the final phase is sharded by node-OWNER core,
   so x2 gathers read the core's local 6272-row shard - no second AllGather
   and single-stream int16 indices.
 - Final phase: batched dma_gathers of x2[node] and effect_emb[effect] rows
   (bf16), one DVE add per group, partition-major fp32 output tiles DMA'd
   contiguously; host unsorts rows.
"""
import sys
sys.path.insert(0, "/opt/trn_rl_repo")
import heapq
import numpy as np

import jax
from jax.sharding import Mesh, PartitionSpec
from jax.experimental.shard_map import shard_map

from concourse import bass, mybir, library_config
from concourse.tile import TileContext
from concourse.bass2jax import (
    _bass_exec_p,
    install_neuronx_cc_hook,
    partition_id_tensor,
)

import ml_dtypes

P = 128
F32 = mybir.dt.float32
BF16 = mybir.dt.bfloat16
I32 = mybir.dt.int32
I16 = mybir.dt.int16
BF = ml_dtypes.bfloat16
HALF = 32768

CFG = dict(N=50000, E=600000, D=128, NEFF=1000, Q=200000, C=8, B=49, HOPS=2)

KG = 64   # chunks per hop gather group (KG*128 rows per dma_gather)
KF = 32   # chunks per final-phase gather group


# ------------------------------------------------------------------ wait split

def _split_wide_waits(nc, max_waits=1):
    """This walrus build rejects instructions with more than one sync-wait
    command; move excess waits onto preceding NoOps on the same engine."""
    for f in nc.m.functions:
        for bb in f.blocks:
            new_instrs = []
            for ins in bb.instructions:
                si = ins.sync_info
                if si is not None and si.on_wait and len(si.on_wait) > max_waits:
                    waits = list(si.on_wait)
                    head, tail = waits[:-max_waits], waits[-max_waits:]
                    for i in range(0, len(head), max_waits):
                        nop = mybir.InstNoOp(
                            name=nc.get_next_instruction_name(),
                            engine=ins.engine,
                            ins=[], outs=[],
                            sync_info=mybir.SyncInfo(
                                on_wait=head[i:i + max_waits], on_update=[]),
                            text_hint="waitsplit",
                            bass_nofuse=True,
                        )
                        new_instrs.append(nop)
                    si.on_wait = tail
                new_instrs.append(ins)
            bb.instructions = new_instrs


# ------------------------------------------------------------------ host prep

def _balance_nodes(dst, N, n_bins):
    npad = n_bins * P
    deg = np.bincount(dst, minlength=N).astype(np.int64)
    deg_pad = np.zeros(npad, np.int64)
    deg_pad[:N] = deg
    order = np.argsort(-deg_pad, kind="stable")
    heap = [(0, b) for b in range(n_bins)]
    heapq.heapify(heap)
    counts = np.zeros(n_bins, np.int32)
    newid = np.empty(npad, np.int64)
    for n in order:
        while True:
            load, b = heapq.heappop(heap)
            if counts[b] < P:
                break
        newid[n] = b * P + counts[b]
        counts[b] += 1
        if counts[b] < P:
            heapq.heappush(heap, (load + deg_pad[n], b))
    assert counts.min() == counts.max() == P
    return newid


def _wrap_idx16(idx, nch):
    """idx[i] -> [16g + i%16, i//16] for g in 0..7 (replica per Q7 core).
    Returns [P, nch*8] int16."""
    ni = nch * P
    full = np.zeros(ni, np.int16)
    full[:len(idx)] = idx
    arr = np.zeros((P, ni // 16), np.int16)
    blk = full.reshape(ni // 16, 16).T
    for g in range(8):
        arr[16 * g:16 * (g + 1)] = blk
    return arr


def _prep(inputs, cfg):
    N, E, D, NEFF, Q = cfg["N"], cfg["E"], cfg["D"], cfg["NEFF"], cfg["Q"]
    C, B = cfg["C"], cfg["B"]
    NPC = B * P
    NPAD = C * NPC
    assert NPAD >= N and D == 128

    graph_x = np.asarray(inputs["graph_x"], np.float32)
    edge_index = np.asarray(inputs["edge_index"])
    src = edge_index[0].astype(np.int64)
    dst = edge_index[1].astype(np.int64)
    w = np.asarray(inputs["chemical_similarity"], np.float32)
    x_nodes = np.asarray(inputs["x_nodes"]).astype(np.int64)
    effect_ids = np.asarray(inputs["effect_ids"]).astype(np.int64)
    W_self = np.asarray(inputs["W_self"], np.float32)
    W_neigh = np.asarray(inputs["W_neigh"], np.float32)
    bias = np.asarray(inputs["bias"], np.float32)
    effect_emb = np.asarray(inputs["effect_emb"], np.float32)

    newid = _balance_nodes(dst, N, C * B)

    x_full = np.zeros((NPAD, D), np.float32)
    x_full[newid[:N]] = graph_x

    # pre-normalized edge weights (reference: agg/max(segsum(w,dst),1e-12))
    denom = np.zeros(N, np.float64)
    np.add.at(denom, dst, w.astype(np.float64))
    wn = (w.astype(np.float64)
          / np.maximum(denom[dst], 1e-12)).astype(np.float32)

    nsrc = newid[src]
    ndst = newid[dst]
    ecore = ndst // NPC
    eblk = (ndst % NPC) // P
    eslot = ndst % P
    ehalf = (nsrc >= HALF).astype(np.int64)

    loads = np.zeros((C, B, 2), np.int64)
    np.add.at(loads, (ecore, eblk, ehalf), 1)
    # uniform chunk counts per (block, stream) across cores
    CS_b = -(-loads.max(axis=0) // P)          # [B, 2]
    zero = CS_b.sum(axis=1) == 0
    CS_b[zero, 0] = 1
    C_b = CS_b.sum(axis=1)                      # chunks per block
    NCH = int(C_b.sum())
    cob = np.concatenate([[0], np.cumsum(C_b)])
    # stream-position bases per block
    sposA0 = np.concatenate([[0], np.cumsum(CS_b[:, 0])])
    sposB0 = np.concatenate([[0], np.cumsum(CS_b[:, 1])])
    NCHA, NCHB = int(sposA0[-1]), int(sposB0[-1])

    # per-chunk metadata (same for all cores): stream + stream position
    chunk_stream = np.zeros(NCH, np.int64)
    chunk_spos = np.zeros(NCH, np.int64)
    for b in range(B):
        ca, cbn = int(CS_b[b, 0]), int(CS_b[b, 1])
        j0 = int(cob[b])
        chunk_stream[j0:j0 + ca] = 0
        chunk_spos[j0:j0 + ca] = sposA0[b] + np.arange(ca)
        chunk_stream[j0 + ca:j0 + ca + cbn] = 1
        chunk_spos[j0 + ca:j0 + ca + cbn] = sposB0[b] + np.arange(cbn)

    edstf = np.zeros((C, P, NCH), np.float32)
    ewf = np.zeros((C, P, NCH), np.float32)
    idxA = np.zeros((C, NCHA * P), np.int16)
    idxB = np.zeros((C, NCHB * P), np.int16)

    # vectorized packing: edges sorted by (core, block, half), rank in group
    eorder = np.lexsort((ehalf, eblk, ecore))
    ec, eb, es = ecore[eorder], eblk[eorder], ehalf[eorder]
    gstart = np.concatenate(
        [[0], np.cumsum(loads.ravel())[:-1]]).reshape(C, B, 2)
    rank = np.arange(E, dtype=np.int64) - gstart[ec, eb, es]
    cig = rank // P                             # chunk index within group
    slot = rank % P
    jglob = cob[eb] + np.where(es == 0, cig, CS_b[eb, 0] + cig)
    spos = np.where(es == 0, sposA0[eb] + cig, sposB0[eb] + cig)
    edstf[ec, slot, jglob] = eslot[eorder].astype(np.float32)
    ewf[ec, slot, jglob] = wn[eorder]
    sidx = spos * P + slot
    vals = (nsrc[eorder] - es * HALF).astype(np.int16)
    mA = es == 0
    idxA[ec[mA], sidx[mA]] = vals[mA]
    idxB[ec[~mA], sidx[~mA]] = vals[~mA]

    # queries sharded by node-owner core
    qnew = newid[x_nodes]
    qc = (qnew // NPC).astype(np.int64)
    qrow = (qnew % NPC).astype(np.int64)
    nq = np.bincount(qc, minlength=C)
    QCH = int(-(-nq.max() // P))
    qnode = np.zeros((C, QCH * P), np.int16)
    qeff = np.zeros((C, QCH * P), np.int16)
    qpos = []
    qorder = np.lexsort((np.arange(Q), qc))
    qstart = np.concatenate([[0], np.cumsum(nq)])
    for c in range(C):
        sel = qorder[qstart[c]:qstart[c + 1]]
        qpos.append(sel)
        qnode[c, :len(sel)] = qrow[sel].astype(np.int16)
        qeff[c, :len(sel)] = effect_ids[sel].astype(np.int16)

    iF = np.tile(np.arange(P, dtype=BF)[None, :], (P, 1))
    ident = np.eye(P, dtype=BF)

    meta = dict(cfg, NPC=NPC, NPAD=NPAD, NCH=NCH, NCHA=NCHA, NCHB=NCHB,
                QCH=QCH, C_b=C_b, CS_b=CS_b, chunk_of_block=cob,
                chunk_stream=chunk_stream, chunk_spos=chunk_spos,
                qpos=qpos, nq=nq)

    x0b = x_full.astype(BF)
    in_maps = []
    for c in range(C):
        in_maps.append({
            "x0b": x0b,
            "xT0": x_full[c * NPC:(c + 1) * NPC].T.astype(BF).copy(),
            "effb": effect_emb.astype(BF),
            "idxA": _wrap_idx16(idxA[c], NCHA),
            "idxB": _wrap_idx16(idxB[c], max(1, NCHB)),
            "edst": edstf[c], "ewn": ewf[c],
            "qnode16": _wrap_idx16(qnode[c], QCH),
            "qeff16": _wrap_idx16(qeff[c], QCH),
            "iF": iF, "ident": ident,
            "Wsb": W_self.astype(BF), "Wnb": W_neigh.astype(BF),
            "biasc": bias.T.astype(np.float32).copy(),
        })
    return meta, in_maps


# --------------------------------------------------------------- device build

def _build_nc(meta, krep=1, no_collective=False):
    C, B, D, NEFF = meta["C"], meta["B"], meta["D"], meta["NEFF"]
    NPC, NPAD, NCH = meta["NPC"], meta["NPAD"], meta["NCH"]
    NCHA, NCHB, QCH = meta["NCHA"], meta["NCHB"], meta["QCH"]
    C_b, CS_b, cob = meta["C_b"], meta["CS_b"], meta["chunk_of_block"]
    chunk_stream, chunk_spos = meta["chunk_stream"], meta["chunk_spos"]
    HOPS = meta["HOPS"]
    NCHBp = max(1, NCHB)

    nc = bass.Bass(trn_type="TRN2", num_devices=C, num_swdge_queues=2)

    x0b = nc.dram_tensor("x0b", [NPAD, D], BF16, kind="ExternalInput")
    xT0 = nc.dram_tensor("xT0", [D, NPC], BF16, kind="ExternalInput")
    effb = nc.dram_tensor("effb", [NEFF, D], BF16, kind="ExternalInput")
    idxA = nc.dram_tensor("idxA", [P, NCHA * 8], I16, kind="ExternalInput")
    idxB = nc.dram_tensor("idxB", [P, NCHBp * 8], I16, kind="ExternalInput")
    edst = nc.dram_tensor("edst", [P, NCH], F32, kind="ExternalInput")
    ewn = nc.dram_tensor("ewn", [P, NCH], F32, kind="ExternalInput")
    qnode16 = nc.dram_tensor("qnode16", [P, QCH * 8], I16,
                             kind="ExternalInput")
    qeff16 = nc.dram_tensor("qeff16", [P, QCH * 8], I16,
                            kind="ExternalInput")
    iFd = nc.dram_tensor("iF", [P, P], BF16, kind="ExternalInput")
    identd = nc.dram_tensor("ident", [P, P], BF16, kind="ExternalInput")
    Wsb = nc.dram_tensor("Wsb", [HOPS, D, D], BF16, kind="ExternalInput")
    Wnb = nc.dram_tensor("Wnb", [HOPS, D, D], BF16, kind="ExternalInput")
    biasc = nc.dram_tensor("biasc", [D, HOPS], F32, kind="ExternalInput")

    newx = [nc.dram_tensor(f"newx{h}", [NPC, D], BF16) for h in range(HOPS)]
    x1_full = nc.dram_tensor("x1_full", [NPAD, D], BF16, addr_space="Shared")
    out_pm = nc.dram_tensor("out_pm", [P, QCH, D], F32, kind="ExternalOutput")

    rg = [list(range(C))]
    gq = [0]
    ni_regs = {}

    def ni_reg(ni):
        if ni not in ni_regs:
            ni_regs[ni] = nc.gpsimd.to_reg(ni)
        return ni_regs[ni]

    def gath(pool, table_ap, idx_sb, spos0, nco, kmax, tag):
        """One dma_gather of nco chunks (nco*128 rows) starting at stream
        position spos0; idx slice cols [spos0*8, (spos0+nco)*8)."""
        t = pool.tile([P, kmax, D], BF16, tag=tag)
        ni = nco * P
        nc.gpsimd.dma_gather(
            t[:, :nco, :], table_ap,
            idx_sb[:, spos0 * 8:(spos0 + nco) * 8],
            ni, ni_reg(ni), D, elem_step=D,
            queue_num=gq[0] % 2, single_packet=False)
        gq[0] += 1
        return t

    with TileContext(nc) as tc:
        nc.gpsimd.load_library(library_config.mlp)
        with tc.tile_pool(name="const", bufs=1) as cp:
            iF = cp.tile([P, P], BF16)
            nc.sync.dma_start(out=iF[:], in_=iFd[:, :])
            idn = cp.tile([P, P], BF16)
            nc.sync.dma_start(out=idn[:], in_=identd[:, :])
            Ws, Wn = [], []
            for h in range(HOPS):
                t = cp.tile([P, D], BF16, tag=f"ws{h}")
                nc.sync.dma_start(out=t[:], in_=Wsb[h, :, :])
                Ws.append(t)
                t = cp.tile([P, D], BF16, tag=f"wn{h}")
                nc.sync.dma_start(out=t[:], in_=Wnb[h, :, :])
                Wn.append(t)
            bc = cp.tile([P, HOPS], F32)
            nc.sync.dma_start(out=bc[:], in_=biasc[:, :])
            edst_sb = cp.tile([P, NCH], F32)
            nc.sync.dma_start(out=edst_sb[:], in_=edst[:, :])
            ewn_sb = cp.tile([P, NCH], F32)
            nc.sync.dma_start(out=ewn_sb[:], in_=ewn[:, :])
            idxA_sb = cp.tile([P, NCHA * 8], I16, tag="idxA")
            nc.sync.dma_start(out=idxA_sb[:], in_=idxA[:, :])
            idxB_sb = cp.tile([P, NCHBp * 8], I16, tag="idxB")
            nc.sync.dma_start(out=idxB_sb[:], in_=idxB[:, :])
            qnode_sb = cp.tile([P, QCH * 8], I16, tag="qn")
            nc.sync.dma_start(out=qnode_sb[:], in_=qnode16[:, :])
            qeff_sb = cp.tile([P, QCH * 8], I16, tag="qe")
            nc.sync.dma_start(out=qeff_sb[:], in_=qeff16[:, :])
            xT0_sb = cp.tile([P, NPC], BF16, tag="xT0")
            nc.sync.dma_start(out=xT0_sb[:], in_=xT0[:, :])
            xT1_sb = cp.tile([P, NPC], BF16, tag="xT1")

            for rep in range(krep):
                for h in range(HOPS):
                    table = x0b if h == 0 else x1_full
                    tabA = table[:HALF, :]
                    tabB = table[HALF:, :]
                    xT_cur = xT0_sb if h == 0 else xT1_sb
                    with tc.tile_pool(name=f"ga{h}_{rep}", bufs=2) as gpa, \
                         tc.tile_pool(name=f"gb{h}_{rep}", bufs=2) as gpb, \
                         tc.tile_pool(name=f"m{h}_{rep}", bufs=8) as mp, \
                         tc.tile_pool(name=f"s{h}_{rep}", bufs=4) as sp, \
                         tc.tile_pool(name=f"pa{h}_{rep}", bufs=2,
                                      space="PSUM") as pa, \
                         tc.tile_pool(name=f"ph{h}_{rep}", bufs=2,
                                      space="PSUM") as ph:
                        gtile = [None, None]
                        gcur = [-1, -1]
                        for b in range(B):
                            nchunks = int(C_b[b])
                            agg_ps = pa.tile([P, P], F32, tag="agg",
                                             space="PSUM")
                            for k in range(nchunks):
                                j = int(cob[b]) + k
                                s = int(chunk_stream[j])
                                spos = int(chunk_spos[j])
                                g = spos // KG
                                if g != gcur[s]:
                                    gcur[s] = g
                                    if s == 0:
                                        nco = min(KG, NCHA - g * KG)
                                        gtile[0] = gath(
                                            gpa, tabA, idxA_sb, g * KG,
                                            nco, KG, "msgA")
                                    else:
                                        nco = min(KG, NCHB - g * KG)
                                        gtile[1] = gath(
                                            gpb, tabB, idxB_sb, g * KG,
                                            nco, KG, "msgB")
                                msg = gtile[s][:, spos - g * KG, :]
                                mask = mp.tile([P, P], BF16, tag="mask")
                                nc.vector.tensor_scalar(
                                    out=mask[:], in0=iF[:],
                                    scalar1=edst_sb[:, j:j + 1],
                                    scalar2=ewn_sb[:, j:j + 1],
                                    op0=mybir.AluOpType.is_equal,
                                    op1=mybir.AluOpType.mult)
                                nc.tensor.matmul(
                                    agg_ps[:], lhsT=msg, rhs=mask[:],
                                    start=(k == 0), stop=(k == nchunks - 1))
                            aggT = sp.tile([P, P], BF16, tag="aggT")
                            nc.scalar.activation(
                                aggT[:], agg_ps[:],
                                mybir.ActivationFunctionType.Copy)
                            hx_ps = ph.tile([P, P], F32, tag="hx",
                                            space="PSUM")
                            nc.tensor.matmul(
                                hx_ps[:], lhsT=Ws[h][:],
                                rhs=xT_cur[:, b * P:(b + 1) * P],
                                start=True, stop=False)
                            nc.tensor.matmul(hx_ps[:], lhsT=Wn[h][:],
                                             rhs=aggT[:],
                                             start=False, stop=True)
                            if h == 0:
                                fm_dst = xT1_sb[:, b * P:(b + 1) * P]
                            else:
                                fm_t = sp.tile([P, P], BF16, tag="fm")
                                fm_dst = fm_t[:]
                            nc.scalar.activation(
                                fm_dst, hx_ps[:],
                                mybir.ActivationFunctionType.Relu,
                                bias=bc[:, h:h + 1])
                            nx_ps = ph.tile([P, P], BF16, tag="nxp",
                                            space="PSUM")
                            nc.tensor.transpose(out=nx_ps[:], in_=fm_dst,
                                                identity=idn[:])
                            nx = sp.tile([P, P], BF16, tag="nx")
                            nc.scalar.activation(
                                nx[:], nx_ps[:],
                                mybir.ActivationFunctionType.Copy)
                            nc.sync.dma_start(
                                out=newx[h][b * P:(b + 1) * P, :], in_=nx[:])
                    if h == 0:
                        if no_collective:
                            nc.sync.dma_start(
                                out=x1_full[0:NPC, :], in_=newx[0][:, :])
                        else:
                            nc.gpsimd.collective_compute(
                                "AllGather", mybir.AluOpType.bypass,
                                replica_groups=rg,
                                ins=[newx[0][:]],
                                outs=[x1_full[:]])

                with tc.tile_pool(name=f"fx_{rep}", bufs=2) as fx, \
                     tc.tile_pool(name=f"fe_{rep}", bufs=2) as fe, \
                     tc.tile_pool(name=f"fo_{rep}", bufs=2) as fo:
                    for g0 in range(0, QCH, KF):
                        nco = min(KF, QCH - g0)
                        xt = gath(fx, newx[HOPS - 1][:, :], qnode_sb, g0,
                                  nco, KF, "x2g")
                        et = gath(fe, effb[:, :], qeff_sb, g0,
                                  nco, KF, "efg")
                        ot = fo.tile([P, KF, D], F32, tag="ot")
                        nc.vector.tensor_tensor(
                            out=ot[:, :nco, :], in0=xt[:, :nco, :],
                            in1=et[:, :nco, :],
                            op=mybir.AluOpType.add)
                        nc.sync.dma_start(
                            out=out_pm[:, g0:g0 + nco, :],
                            in_=ot[:, :nco, :])
    return nc


# ------------------------------------------------------------------- runner

def _build_runner(nc, n_cores):
    install_neuronx_cc_hook()
    partition_name = nc.partition_id_tensor.name if nc.partition_id_tensor else None

    in_names, out_names, out_avals = [], [], []
    for alloc in nc.m.functions[0].allocations:
        if not isinstance(alloc, mybir.MemoryLocationSet):
            continue
        name = alloc.memorylocations[0].name
        if alloc.kind == "ExternalInput":
            if name != partition_name:
                in_names.append(name)
        elif alloc.kind == "ExternalOutput":
            out_names.append(name)
            out_avals.append(jax.core.ShapedArray(
                tuple(alloc.tensor_shape), mybir.dt.np(alloc.dtype)))

    n_params = len(in_names)
    n_outs = len(out_avals)
    all_in_names = list(in_names) + list(out_names)
    if partition_name is not None:
        all_in_names.append(partition_name)

    def _body(*args):
        operands = list(args)
        if partition_name is not None:
            operands.append(partition_id_tensor())
        outs = _bass_exec_p.bind(
            *operands,
            out_avals=tuple(out_avals),
            in_names=tuple(all_in_names),
            out_names=tuple(out_names),
            lowering_input_output_aliases=(),
            sim_require_finite=True,
            sim_require_nnan=True,
            nc=nc,
        )
        return tuple(outs)

    devices = jax.devices()[:n_cores]
    mesh = Mesh(np.asarray(devices), ("core",))
    in_specs = (PartitionSpec("core"),) * (n_params + n_outs)
    out_specs = (PartitionSpec("core"),) * n_outs
    sharded = jax.jit(
        shard_map(_body, mesh=mesh, in_specs=in_specs, out_specs=out_specs,
                  check_rep=False),
        keep_unused=True,
    )

    def make_args(in_maps):
        per_core = [[np.asarray(m[name]) for name in in_names] for m in in_maps]
        concat_in = [
            np.concatenate([per_core[c][i] for c in range(n_cores)], axis=0)
            for i in range(n_params)
        ]
        concat_zeros = [
            np.zeros((n_cores * av.shape[0], *av.shape[1:]), av.dtype)
            for av in out_avals
        ]
        from jax.sharding import NamedSharding
        sh = NamedSharding(mesh, PartitionSpec("core"))
        return [jax.device_put(a, sh) for a in concat_in + concat_zeros]

    def run(in_maps):
        args = make_args(in_maps)
        out_arrs = sharded(*args)
        jax.block_until_ready(out_arrs)
        return [
            {name: np.asarray(out_arrs[i]).reshape(
                n_cores, *out_avals[i].shape)[c]
             for i, name in enumerate(out_names)}
            for c in range(n_cores)
        ]

    def timeit(in_maps, reps=1):
        import time
        args = make_args(in_maps)
        out = sharded(*args)
        jax.block_until_ready(out)
        ts = []
        for _ in range(reps):
            t0 = time.perf_counter()
            out = sharded(*args)
            jax.block_until_ready(out)
            ts.append(time.perf_counter() - t0)
        return out, ts

    return run, timeit


# ------------------------------------------------------------------- kernel

def kernel(**inputs):
    gx = np.asarray(inputs["graph_x"])
    cfg = dict(
        N=gx.shape[0],
        E=np.asarray(inputs["edge_index"]).shape[1],
        D=gx.shape[1],
        NEFF=np.asarray(inputs["effect_emb"]).shape[0],
        Q=np.asarray(inputs["x_nodes"]).shape[0],
        C=8,
        B=-(-gx.shape[0] // (8 * P)),
        HOPS=np.asarray(inputs["W_self"]).shape[0],
    )
    meta, in_maps = _prep(inputs, cfg)
    nc = _build_nc(meta)
    _split_wide_waits(nc, 1)
    mybir.codegen_inst_isa_subclasses(nc)
    run, _ = _build_runner(nc, cfg["C"])
    results = run(in_maps)

    C, D, Q, QCH = cfg["C"], cfg["D"], cfg["Q"], meta["QCH"]
    out = np.empty((Q, D), np.float32)
    for c in range(C):
        pm = results[c]["out_pm"]                       # [P, QCH, P]
        vals = pm.transpose(1, 0, 2).reshape(QCH * P, D)
        sel = meta["qpos"][c]
        out[sel] = vals[:len(sel)]
    return out
